# revision 38
# baseline (speedup 1.0000x reference)
"""Quantized 3x3 conv (8-bit symmetric STE quantization of x and w, then
stride-1 pad-1 conv) on 8 Trainium2 NeuronCores.

Strategy
--------
Data-parallel over batch: 4 images per core (32/8).  The quantization is
integer-exact, so it is hoisted to the host:
  * x is quantized host-side to integers kx in [-127,127] (reproducing
    jnp.round(x/step) bit-exactly), packed into zero-padded 58x58 bf16
    grids, and DMA'd directly into the matmul operand layout.
  * w is quantized host-side, laid out as lhsT [ci, tap, co] bf16 and
    duplicated into both partition halves.
Per core:
  * conv = 9 shifted matmuls (K=ci=64, M=co=128) accumulating in PSUM.
    Two images run concurrently on the PE via row-group tiling: image (2g)
    on partitions 0-63, image (2g+1) on partitions 64-127, so the full
    128-row array streams both images' columns concurrently.  This is the
    per-core PE roofline (~23.5 us of column streaming at 2.4 GHz).
  * PE warmup matmul PAIRS (both row groups) bridge the DMA head so the
    HAM clock monitor sees full-array activity and un-throttles the PE
    clock (1.2 -> 2.4 GHz) at the earliest ~3.4 us window.  Half-row
    warmups measurably do NOT count as busy.
  * input DMAs are chunked by grid rows with boundaries aligned to the
    8-row output blocks, so the early (cold-clock) entries never stall on
    a late fat chunk.
  * PSUM -> SBUF copy applies the final scale s2 = step_x*step_w and
    stores bf16; outputs DMA back at block-group boundaries, even images
    on the SP queue, odd images on the ACT queue (~0.6 us flat issue cost
    per DMA instruction makes one-issue-per-engine tails optimal), and
    are upcast to fp32 on the host.
"""

import numpy as np
import ml_dtypes

import concourse.bass as bass
import concourse.mybir as mybir
import concourse.tile as tile
from concourse import bacc
from concourse.bass_utils import run_bass_kernel_spmd

dt = mybir.dt

N_CORES = 8
NPC = 4                # images per core
CI, CO = 64, 128
H = W = 56
WP = 58                # padded row width (56 + 2)
LEAD = 4               # guard elems before the padded grid
IMG_ELEMS = LEAD + WP * WP + 8   # 4 + 3364 + 8 = 3376
PACK = H * W           # 3136
H0S = [1 + 8 * i for i in range(7)]   # padded-row start of each 8-row block
N_WARM = 25            # PE warmup matmul pairs (HAM clock-ramp bridge); a few
                       # pairs of margin past the expected first-chunk arrival
                       # so a warmup->data gap never breaks the HAM busy
                       # window (a broken window costs ~0.5*3.4 us expected,
                       # the margin costs ~0.15 us guaranteed).

# grid-row boundaries of the input DMA chunks.  Pair-0 boundaries align with
# block needs: block 0 reads grid rows 0-9 (chunk 0), block 1 rows 8-17
# (chunks 0-1), block 2 rows 16-25 (chunks 1-2) -- so entries 0 and 1 never
# stall on a late fat chunk (measured ~0.65 us of early pacing stalls with
# the old [0,9,12,26,...] cut).
RB0 = [0, 10, 18, 26, 42, 58]  # pair 0: small first chunks (block 0 unblocks)
RB1 = [0, 18, 42, 58]          # pair 1: latency-insensitive
ITERS = [[0], [1, 2], [3, 4], [5], [6]]
OUT_CHUNKS = {1: (0, 3), 2: (3, 5), 3: (5, 6), 4: (6, 7)}

_PROG_CACHE = {}


def _chunk_cols(rb, ci):
    c0 = 0 if ci == 0 else LEAD + WP * rb[ci]
    c1 = IMG_ELEMS if ci == len(rb) - 2 else LEAD + WP * rb[ci + 1]
    return c0, c1


def _build_program(s2):
    s2 = float(np.float32(s2))
    nc = bacc.Bacc(None)
    x_in = nc.declare_dram_parameter("xg", [2 * 128, IMG_ELEMS], dt.bfloat16,
                                     isOutput=False)
    wq_in = nc.declare_dram_parameter("wq", [128, 9, CO], dt.bfloat16,
                                      isOutput=False)
    out = nc.declare_dram_parameter("out", [NPC * CO, PACK], dt.bfloat16,
                                    isOutput=True)

    with tile.TileContext(nc) as tc:
        with (
            tc.tile_pool(name="sb", bufs=1) as sb,
            tc.tile_pool(name="ps", bufs=4, space="PSUM") as psp,
        ):
            wq = sb.tile([128, 9, CO], dt.bfloat16)
            dummy = sb.tile([128, 128], dt.bfloat16)
            xq = [sb.tile([128, IMG_ELEMS], dt.bfloat16, name=f"xq{g}", tag=f"xq{g}")
                  for g in range(2)]
            os_ = [sb.tile([128, PACK], dt.bfloat16, name=f"os{n}", tag=f"os{n}")
                   for n in range(NPC)]

            # warmup fodder must be initialized before the PE touches it
            nc.gpsimd.memset(dummy[:], 0.0)

            def x_dma(eng, g, rb, ci, split=False):
                c0, c1 = _chunk_cols(rb, ci)
                if split:
                    eng.dma_start(out=xq[g][64:128, c0:c1],
                                  in_=x_in[128 * g + 64:128 * (g + 1), c0:c1])
                    eng.dma_start(out=xq[g][0:64, c0:c1],
                                  in_=x_in[128 * g:128 * g + 64, c0:c1])
                else:
                    eng.dma_start(out=xq[g][:, c0:c1],
                                  in_=x_in[128 * g:128 * (g + 1), c0:c1])

            nc.scalar.dma_start(out=wq[:, 0:1, :], in_=wq_in[:, 0:1, :])
            nc.scalar.dma_start(out=wq[:, 1:5, :], in_=wq_in[:, 1:5, :])
            x_dma(nc.sync, 0, RB0, 0, split=True)
            x_dma(nc.sync, 0, RB0, 1)
            x_dma(nc.sync, 0, RB0, 2)
            nc.sync.dma_start(out=wq[:, 5:9, :], in_=wq_in[:, 5:9, :])
            for ci in range(3, len(RB0) - 1):
                x_dma(nc.sync, 0, RB0, ci)
            for ci in range(len(RB1) - 1):
                x_dma(nc.sync, 1, RB1, ci)

            # PE warmup (HAM clock-ramp) overlapping the DMA head.  Issued as
            # row-group PAIRS (h0 + h64, like the real matmuls) so the full
            # 128-row array is active: half-row warmups were measured NOT to
            # trip the HAM busy detector (fire tracked the first REAL matmul
            # + ~5.3 us, not the warmup start).
            warm = psp.tile([128, 512], dt.float32, name="warm", tag="ps")
            warm2 = psp.tile([128, 512], dt.float32, name="warm2", tag="ps")
            for _ in range(N_WARM):
                nc.tensor.matmul(
                    warm[:, 0:128], lhsT=dummy[64:128, 0:128],
                    rhs=dummy[64:128, 0:128], start=True, stop=True,
                )
                nc.tensor.matmul(
                    warm2[:, 0:128], lhsT=dummy[0:64, 0:128],
                    rhs=dummy[0:64, 0:128], start=True, stop=True,
                )
            nc.vector.tensor_copy(os_[0][0:1, 0:1], warm[0:1, 0:1])
            nc.vector.tensor_copy(os_[0][0:1, 1:2], warm2[0:1, 0:1])

            for g in range(2):
                entries = [
                    (0, 1, 0, 8, None),
                    (1, 2, 0, 8, (0, 1344)),
                    (3, 2, 0, 8, (1344, 2240)),
                    (5, 1, 0, 8, (2240, 2688)),
                ]
                if g == 0:
                    entries.append((6, 1, 0, 8, (2688, 3136)))
                else:
                    entries.append((6, 1, 0, 4, (2688, 2912)))
                    entries.append((6, 1, 4, 8, (2912, 3136)))
                for b0, nb, q0, q1, oc in entries:
                    nr = q1 - q0
                    ps_pair = [psp.tile([128, 1024], dt.float32,
                                        name=f"psum_g{g}b{b0}q{q0}h{h}", tag="ps")
                               for h in range(2)]
                    ps2 = [p.rearrange("p (b x) -> p b x", b=2) for p in ps_pair]
                    for t in range(9):
                        dh, dw = t // 3, t % 3
                        for h in (1, 0):
                            for bi in range(nb):
                                off = (LEAD + (H0S[b0 + bi] + dh - 1 + q0) * WP
                                       + dw)
                                rhs = xq[g][64 * h:64 * (h + 1),
                                            off:off + nr * WP].rearrange(
                                    "p (r c) -> p r c", c=WP)[:, :, 0:56]
                                nc.tensor.matmul(
                                    ps2[h][:, bi, 0:56 * nr],
                                    lhsT=wq[64 * h:64 * (h + 1), t, :],
                                    rhs=rhs,
                                    start=(t == 0), stop=(t == 8),
                                )
                    for h in range(2):
                        img = 2 * g + h
                        sel = ps2[h][:, 0:nb, 0:56 * nr]
                        dst = os_[img][:, 448 * b0 + 56 * q0:
                                       448 * (b0 + nb - 1) + 56 * q1]
                        if h == 0:
                            nc.vector.tensor_scalar_mul(
                                out=dst, in0=sel, scalar1=s2)
                        else:
                            nc.scalar.activation(
                                out=dst, in_=sel,
                                func=mybir.ActivationFunctionType.Copy,
                                scale=s2,
                            )
                    # output DMA at boundaries; even images on the SP queue,
                    # odd images on the ACT queue (parallel descriptor issue).
                    # (DMA issue is ~0.6 us flat per instruction; rebalancing
                    # the tail issues onto one queue was measured WORSE --
                    # three serialized issues pile up behind the last scale.)
                    if oc is not None:
                        for h in range(2):
                            img = 2 * g + h
                            eng = nc.sync if h == 0 else nc.scalar
                            eng.dma_start(
                                out=out[CO * img:CO * (img + 1), oc[0]:oc[1]],
                                in_=os_[img][:, oc[0]:oc[1]],
                            )
    if not nc.is_finalized():
        nc.finalize()
    return nc


def _host_prep(x, w, alpha_x, alpha_w):
    x = np.asarray(x, dtype=np.float32)
    w = np.asarray(w, dtype=np.float32)
    ax = np.float32(max(np.float32(np.asarray(alpha_x).reshape(-1)[0]), np.float32(0)))
    aw = np.float32(max(np.float32(np.asarray(alpha_w).reshape(-1)[0]), np.float32(0)))
    step_x = np.float32(np.float32(np.float32(2.0) * ax) / np.float32(254.0))
    step_w = np.float32(np.float32(np.float32(2.0) * aw) / np.float32(254.0))
    s2 = np.float32(step_x * step_w)

    with np.errstate(divide="ignore", invalid="ignore"):
        kx = np.clip(np.round(x / step_x), -127, 127)
        kw = np.clip(np.round((w / step_w).astype(np.float32)), -127, 127)
    kx = np.nan_to_num(kx, nan=0.0, posinf=127.0, neginf=-127.0)

    n = x.shape[0]
    grids = np.zeros((n, CI, IMG_ELEMS), dtype=ml_dtypes.bfloat16)
    gv = grids[:, :, LEAD:LEAD + WP * WP].reshape(n, CI, WP, WP)
    gv[:, :, 1:57, 1:57] = kx.reshape(n, CI, H, W)

    kw = np.nan_to_num(kw, nan=0.0, posinf=127.0, neginf=-127.0)
    kw = kw.astype(np.float32).reshape(CO, CI, 9).transpose(1, 2, 0)
    wq = np.concatenate([kw, kw], axis=0).astype(ml_dtypes.bfloat16)
    return grids, wq, s2


def _in_maps(grids, wq):
    return [
        {
            "xg": grids[NPC * c:NPC * (c + 1)].reshape(2 * 128, IMG_ELEMS),
            "wq": wq,
        }
        for c in range(N_CORES)
    ]


def get_program(s2=float(np.float32(np.float32(2.0 / 254.0) ** 2))):
    key = float(np.float32(s2))
    if key not in _PROG_CACHE:
        _PROG_CACHE[key] = _build_program(key)
    return _PROG_CACHE[key]


def run_on_hw(x, w, alpha_x, alpha_w, trace=False):
    grids, wq, s2 = _host_prep(x, w, alpha_x, alpha_w)
    nc = get_program(s2)
    res = run_bass_kernel_spmd(nc, _in_maps(grids, wq),
                               list(range(N_CORES)), trace=trace)
    out = np.concatenate(
        [np.asarray(res.results[i]["out"]).astype(np.float32).reshape(NPC, CO, H, W)
         for i in range(N_CORES)], axis=0)
    return out, res


def kernel(x, w, alpha_x, alpha_w):
    out, _ = run_on_hw(x, w, alpha_x, alpha_w)
    return out


# revision 40
# speedup vs baseline: 1.0728x; 1.0728x over previous
"""Quantized 3x3 conv (8-bit symmetric STE quantization of x and w, then
stride-1 pad-1 conv) on 8 Trainium2 NeuronCores.

Strategy
--------
Data-parallel over batch: 4 images per core (32/8).  The quantization is
integer-exact, so it is hoisted to the host:
  * x is quantized host-side to integers kx in [-127,127] (reproducing
    jnp.round(x/step) bit-exactly), packed into zero-padded 58x58 bf16
    grids, and DMA'd directly into the matmul operand layout.
  * w is quantized host-side, laid out as lhsT [ci, tap, co] bf16 and
    duplicated into both partition halves.
Per core:
  * conv = 9 shifted matmuls (K=ci=64, M=co=128) accumulating in PSUM.
    Two images run concurrently on the PE via row-group tiling: image (2g)
    on partitions 0-63, image (2g+1) on partitions 64-127, so the full
    128-row array streams both images' columns concurrently.  This is the
    per-core PE roofline (~23.5 us of column streaming at 2.4 GHz).
  * PE warmup matmul PAIRS (both row groups) bridge the DMA head so the
    HAM clock monitor sees full-array activity and un-throttles the PE
    clock (1.2 -> 2.4 GHz) at the earliest ~3.4 us window.  Half-row
    warmups measurably do NOT count as busy.
  * input DMAs are chunked by grid rows with boundaries aligned to the
    8-row output blocks, so the early (cold-clock) entries never stall on
    a late fat chunk.
  * PSUM -> SBUF copy applies the final scale s2 = step_x*step_w and
    stores bf16; outputs DMA back at block-group boundaries, even images
    on the SP queue, odd images on the ACT queue (~0.6 us flat issue cost
    per DMA instruction makes one-issue-per-engine tails optimal), and
    are upcast to fp32 on the host.
"""

import numpy as np
import ml_dtypes

import concourse.bass as bass
import concourse.mybir as mybir
import concourse.tile as tile
from concourse import bacc
from concourse.bass_utils import run_bass_kernel_spmd

dt = mybir.dt

N_CORES = 8
NPC = 4                # images per core
CI, CO = 64, 128
H = W = 56
WP = 58                # padded row width (56 + 2)
LEAD = 4               # guard elems before the padded grid
IMG_ELEMS = LEAD + WP * WP + 8   # 4 + 3364 + 8 = 3376
PACK = H * W           # 3136
H0S = [1 + 8 * i for i in range(7)]   # padded-row start of each 8-row block
N_WARM = 25            # PE warmup matmul pairs (HAM clock-ramp bridge); a few
                       # pairs of margin past the expected first-chunk arrival
                       # so a warmup->data gap never breaks the HAM busy
                       # window (a broken window costs ~0.5*3.4 us expected,
                       # the margin costs ~0.15 us guaranteed).

# grid-row boundaries of the input DMA chunks.  Pair-0 boundaries align with
# block needs: block 0 reads grid rows 0-9 (chunk 0), block 1 rows 8-17
# (chunks 0-1), block 2 rows 16-25 (chunks 1-2) -- so entries 0 and 1 never
# stall on a late fat chunk (measured ~0.65 us of early pacing stalls with
# the old [0,9,12,26,...] cut).
RB0 = [0, 10, 18, 26, 42, 58]  # pair 0: small first chunks (block 0 unblocks)
RB1 = [0, 18, 42, 58]          # pair 1: latency-insensitive
ITERS = [[0], [1, 2], [3, 4], [5], [6]]
OUT_CHUNKS = {1: (0, 3), 2: (3, 5), 3: (5, 6), 4: (6, 7)}

_PROG_CACHE = {}


def _chunk_cols(rb, ci):
    c0 = 0 if ci == 0 else LEAD + WP * rb[ci]
    c1 = IMG_ELEMS if ci == len(rb) - 2 else LEAD + WP * rb[ci + 1]
    return c0, c1


def _build_program(s2):
    s2 = float(np.float32(s2))
    nc = bacc.Bacc(None)
    x_in = nc.declare_dram_parameter("xg", [2 * 128, IMG_ELEMS], dt.bfloat16,
                                     isOutput=False)
    wq_in = nc.declare_dram_parameter("wq", [128, 9, CO], dt.bfloat16,
                                      isOutput=False)
    out = nc.declare_dram_parameter("out", [NPC * CO, PACK], dt.bfloat16,
                                    isOutput=True)

    with tile.TileContext(nc) as tc:
        with (
            tc.tile_pool(name="sb", bufs=1) as sb,
            tc.tile_pool(name="ps", bufs=4, space="PSUM") as psp,
        ):
            wq = sb.tile([128, 9, CO], dt.bfloat16)
            dummy = sb.tile([128, 128], dt.bfloat16)
            xq = [sb.tile([128, IMG_ELEMS], dt.bfloat16, name=f"xq{g}", tag=f"xq{g}")
                  for g in range(2)]
            os_ = [sb.tile([128, PACK], dt.bfloat16, name=f"os{n}", tag=f"os{n}")
                   for n in range(NPC)]

            # warmup fodder must be initialized before the PE touches it
            nc.gpsimd.memset(dummy[:], 0.0)

            def x_dma(eng, g, rb, ci, split=False):
                c0, c1 = _chunk_cols(rb, ci)
                if split:
                    eng.dma_start(out=xq[g][64:128, c0:c1],
                                  in_=x_in[128 * g + 64:128 * (g + 1), c0:c1])
                    eng.dma_start(out=xq[g][0:64, c0:c1],
                                  in_=x_in[128 * g:128 * g + 64, c0:c1])
                else:
                    eng.dma_start(out=xq[g][:, c0:c1],
                                  in_=x_in[128 * g:128 * (g + 1), c0:c1])

            nc.scalar.dma_start(out=wq[:, 0:1, :], in_=wq_in[:, 0:1, :])
            nc.scalar.dma_start(out=wq[:, 1:5, :], in_=wq_in[:, 1:5, :])
            x_dma(nc.sync, 0, RB0, 0, split=True)
            x_dma(nc.sync, 0, RB0, 1)
            x_dma(nc.sync, 0, RB0, 2)
            nc.sync.dma_start(out=wq[:, 5:9, :], in_=wq_in[:, 5:9, :])
            for ci in range(3, len(RB0) - 1):
                x_dma(nc.sync, 0, RB0, ci)
            for ci in range(len(RB1) - 1):
                x_dma(nc.sync, 1, RB1, ci)

            # PE warmup (HAM clock-ramp) overlapping the DMA head.  Issued as
            # row-group PAIRS (h0 + h64, like the real matmuls) so the full
            # 128-row array is active: half-row warmups were measured NOT to
            # trip the HAM busy detector (fire tracked the first REAL matmul
            # + ~5.3 us, not the warmup start).
            warm = psp.tile([128, 512], dt.float32, name="warm", tag="ps")
            warm2 = psp.tile([128, 512], dt.float32, name="warm2", tag="ps")
            for _ in range(N_WARM):
                nc.tensor.matmul(
                    warm[:, 0:128], lhsT=dummy[64:128, 0:128],
                    rhs=dummy[64:128, 0:128], start=True, stop=True,
                )
                nc.tensor.matmul(
                    warm2[:, 0:128], lhsT=dummy[0:64, 0:128],
                    rhs=dummy[0:64, 0:128], start=True, stop=True,
                )
            nc.vector.tensor_copy(os_[0][0:1, 0:1], warm[0:1, 0:1])
            nc.vector.tensor_copy(os_[0][0:1, 1:2], warm2[0:1, 0:1])

            for g in range(2):
                entries = [
                    (0, 1, 0, 8, None),
                    (1, 2, 0, 8, (0, 1344)),
                    (3, 2, 0, 8, (1344, 2240)),
                    (5, 1, 0, 8, (2240, 2688)),
                ]
                if g == 0:
                    entries.append((6, 1, 0, 8, (2688, 3136)))
                else:
                    entries.append((6, 1, 0, 4, (2688, 2912)))
                    entries.append((6, 1, 4, 8, (2912, 3136)))
                for b0, nb, q0, q1, oc in entries:
                    nr = q1 - q0
                    ps_pair = [psp.tile([128, 1024], dt.float32,
                                        name=f"psum_g{g}b{b0}q{q0}h{h}", tag="ps")
                               for h in range(2)]
                    ps2 = [p.rearrange("p (b x) -> p b x", b=2) for p in ps_pair]
                    for t in range(9):
                        dh, dw = t // 3, t % 3
                        for h in (1, 0):
                            for bi in range(nb):
                                off = (LEAD + (H0S[b0 + bi] + dh - 1 + q0) * WP
                                       + dw)
                                rhs = xq[g][64 * h:64 * (h + 1),
                                            off:off + nr * WP].rearrange(
                                    "p (r c) -> p r c", c=WP)[:, :, 0:56]
                                nc.tensor.matmul(
                                    ps2[h][:, bi, 0:56 * nr],
                                    lhsT=wq[64 * h:64 * (h + 1), t, :],
                                    rhs=rhs,
                                    start=(t == 0), stop=(t == 8),
                                )
                    for h in range(2):
                        img = 2 * g + h
                        sel = ps2[h][:, 0:nb, 0:56 * nr]
                        dst = os_[img][:, 448 * b0 + 56 * q0:
                                       448 * (b0 + nb - 1) + 56 * q1]
                        if h == 0:
                            nc.vector.tensor_scalar_mul(
                                out=dst, in0=sel, scalar1=s2)
                        else:
                            nc.scalar.activation(
                                out=dst, in_=sel,
                                func=mybir.ActivationFunctionType.Copy,
                                scale=s2,
                            )
                    # output DMA at boundaries; even images on the SP queue,
                    # odd images on the ACT queue (parallel descriptor issue).
                    # (DMA issue is ~0.6 us flat per instruction; rebalancing
                    # the tail issues onto one queue was measured WORSE --
                    # three serialized issues pile up behind the last scale.)
                    if oc is not None:
                        for h in range(2):
                            img = 2 * g + h
                            eng = nc.sync if h == 0 else nc.scalar
                            eng.dma_start(
                                out=out[CO * img:CO * (img + 1), oc[0]:oc[1]],
                                in_=os_[img][:, oc[0]:oc[1]],
                            )
    if not nc.is_finalized():
        nc.finalize()
    return nc


def _host_prep(x, w, alpha_x, alpha_w):
    x = np.asarray(x, dtype=np.float32)
    w = np.asarray(w, dtype=np.float32)
    ax = np.float32(max(np.float32(np.asarray(alpha_x).reshape(-1)[0]), np.float32(0)))
    aw = np.float32(max(np.float32(np.asarray(alpha_w).reshape(-1)[0]), np.float32(0)))
    step_x = np.float32(np.float32(np.float32(2.0) * ax) / np.float32(254.0))
    step_w = np.float32(np.float32(np.float32(2.0) * aw) / np.float32(254.0))
    s2 = np.float32(step_x * step_w)

    with np.errstate(divide="ignore", invalid="ignore"):
        kx = np.clip(np.round(x / step_x), -127, 127)
        kw = np.clip(np.round((w / step_w).astype(np.float32)), -127, 127)
    kx = np.nan_to_num(kx, nan=0.0, posinf=127.0, neginf=-127.0)

    n = x.shape[0]
    grids = np.zeros((n, CI, IMG_ELEMS), dtype=ml_dtypes.bfloat16)
    gv = grids[:, :, LEAD:LEAD + WP * WP].reshape(n, CI, WP, WP)
    gv[:, :, 1:57, 1:57] = kx.reshape(n, CI, H, W)

    kw = np.nan_to_num(kw, nan=0.0, posinf=127.0, neginf=-127.0)
    kw = kw.astype(np.float32).reshape(CO, CI, 9).transpose(1, 2, 0)
    wq = np.concatenate([kw, kw], axis=0).astype(ml_dtypes.bfloat16)
    return grids, wq, s2


def _in_maps(grids, wq):
    return [
        {
            "xg": grids[NPC * c:NPC * (c + 1)].reshape(2 * 128, IMG_ELEMS),
            "wq": wq,
        }
        for c in range(N_CORES)
    ]


def get_program(s2=float(np.float32(np.float32(2.0 / 254.0) ** 2))):
    key = float(np.float32(s2))
    if key not in _PROG_CACHE:
        _PROG_CACHE[key] = _build_program(key)
    return _PROG_CACHE[key]


def run_on_hw(x, w, alpha_x, alpha_w, trace=False):
    grids, wq, s2 = _host_prep(x, w, alpha_x, alpha_w)
    nc = get_program(s2)
    res = run_bass_kernel_spmd(nc, _in_maps(grids, wq),
                               list(range(N_CORES)), trace=trace)
    out = np.concatenate(
        [np.asarray(res.results[i]["out"]).astype(np.float32).reshape(NPC, CO, H, W)
         for i in range(N_CORES)], axis=0)
    return out, res


def kernel(x, w, alpha_x, alpha_w):
    out, _ = run_on_hw(x, w, alpha_x, alpha_w)
    return out
